# revision 6
# baseline (speedup 1.0000x reference)
"""Bidirectional Mamba layer on 8 Trainium2 NeuronCores.

Sharding: core = (batch b in {0,1}) x (direction in {fwd,bwd}) x
(d_inner half in {0,1}).  Each core runs the full front-end (LN,
in_proj, conv, x_proj, dt) and the selective scan + output projection
for its 128 d_inner channels.  The host flips the sequence for the
backward direction, slices weights per core, and sums the 4 partial
(d_model, L) outputs per batch plus the residual.

One SPMD Bass graph serves all 8 cores; all per-core variation lives in
the input data (weight slices / flipped x).
"""

import math
import numpy as np

import concourse.bass as bass
import concourse.bacc as bacc
import concourse.mybir as mybir
from concourse import tile
from concourse.bass_utils import run_bass_kernel_spmd

# Problem shape (hardcoded per contract)
B_SZ = 2
D_MODEL = 128
D_STATE = 16
D_CONV = 4
EXPAND = 2
D_INNER = EXPAND * D_MODEL          # 256
DT_RANK = math.ceil(D_MODEL / 16)   # 8
LN_EPS = 1e-5
SPATIAL = (32, 16, 16)
L = 32 * 16 * 16                    # 8192
EH = 128                            # d_inner half per core
T = 2048                            # time block
NBLK = L // T
NT = T // 512                       # 512-tiles per block

f32 = mybir.dt.float32
f16 = mybir.dt.float16
A_OP = mybir.AluOpType
AF = mybir.ActivationFunctionType

_CACHED_NC = None


def _build_nc():
    nc = bacc.Bacc("TRN2", target_bir_lowering=False, debug=False, num_devices=8)

    # ---- DRAM parameters (per-core data) ----
    x_d = nc.declare_dram_parameter("x", [L, D_MODEL], f32, isOutput=False)
    wconv_d = nc.declare_dram_parameter("wconvT", [128, 2 * D_CONV * 128], f16, isOutput=False)
    wz_d = nc.declare_dram_parameter("wzT", [128, 128], f16, isOutput=False)
    sbz_d = nc.declare_dram_parameter("sbz", [128, 1], f32, isOutput=False)
    convb_d = nc.declare_dram_parameter("convb", [128, 2], f32, isOutput=False)
    wx_d = nc.declare_dram_parameter("wxT", [128, 80], f16, isOutput=False)
    wdt_d = nc.declare_dram_parameter("wdtT", [DT_RANK, 128], f16, isOutput=False)
    bdt_d = nc.declare_dram_parameter("bdt", [128, 1], f32, isOutput=False)
    a_d = nc.declare_dram_parameter("A", [128, D_STATE], f32, isOutput=False)
    dsk_d = nc.declare_dram_parameter("Dskip", [128, 1], f32, isOutput=False)
    wout_d = nc.declare_dram_parameter("woutT", [128, 128], f16, isOutput=False)
    ident_d = nc.declare_dram_parameter("ident", [128, 128], f16, isOutput=False)
    out_d = nc.declare_dram_parameter("out", [D_MODEL, L], f32, isOutput=True)

    with tile.TileContext(nc) as tc:
        with (
            tc.tile_pool(name="const", bufs=1) as cpool,
            tc.tile_pool(name="ln", bufs=3) as lnpool,
            tc.tile_pool(name="fe", bufs=2) as fepool,
            tc.tile_pool(name="scan", bufs=2) as spool,
            tc.tile_pool(name="mm", bufs=3, space="PSUM") as mmpool,
            tc.tile_pool(name="psx", bufs=2, space="PSUM") as psxpool,
            tc.tile_pool(name="dram", bufs=2, space="DRAM") as dpool,
        ):
            # ---- constants ----
            wconv = cpool.tile([128, 2 * D_CONV * 128], f16)
            wz = cpool.tile([128, 128], f16)
            sbz = cpool.tile([128, 1], f32)
            convb = cpool.tile([128, 2], f32)
            wx = cpool.tile([128, 80], f16)
            wdt = cpool.tile([DT_RANK, 128], f16)
            bdt = cpool.tile([128, 1], f32)
            a_t = cpool.tile([128, D_STATE], f32)
            dsk = cpool.tile([128, 1], f32)
            wout = cpool.tile([128, 128], f16)
            ident = cpool.tile([128, 128], f16)
            carry = cpool.tile([128, D_STATE], f32)
            for sb_t, dr in ((wconv, wconv_d), (wz, wz_d), (sbz, sbz_d),
                             (convb, convb_d), (wx, wx_d), (wdt, wdt_d),
                             (bdt, bdt_d), (a_t, a_d), (dsk, dsk_d),
                             (wout, wout_d), (ident, ident_d)):
                nc.sync.dma_start(out=sb_t[:], in_=dr[:])
            nc.vector.memset(carry[:], 0.0)

            prev_xn = None
            for blk in range(NBLK):
                t0 = blk * T
                # ---------- LN + transpose into xn (c-part, 3+T) ----------
                xn = fepool.tile([128, 3 + T], f16, tag="xn")
                if prev_xn is None:
                    nc.vector.memset(xn[:, 0:3], 0.0)
                else:
                    nc.vector.tensor_copy(xn[:, 0:3], prev_xn[:, T:T + 3])
                # LN per 128-t tile. Identity/Square live in every act table;
                # Sqrt keeps the whole phase on one table (no ln/exp thrash).
                for j in range(NT):
                    psx = psxpool.tile([128, 512], f16, tag="psx")
                    for q in range(4):
                        i = j * 4 + q
                        xt = lnpool.tile([128, 128], f32, tag="xt")
                        nc.sync.dma_start(out=xt[:], in_=x_d[t0 + i * 128: t0 + (i + 1) * 128, :])
                        s1 = lnpool.tile([128, 1], f32, tag="s1")
                        scr = lnpool.tile([128, 128], f32, tag="scr")
                        nc.scalar.activation(scr[:], xt[:], AF.Identity, accum_out=s1[:])
                        negm = lnpool.tile([128, 1], f32, tag="negm")
                        nc.vector.tensor_scalar(negm[:], s1[:], -1.0 / 128, None, A_OP.mult)
                        s2 = lnpool.tile([128, 1], f32, tag="s2")
                        nc.scalar.activation(scr[:], xt[:], AF.Square, bias=negm[:], accum_out=s2[:])
                        v = lnpool.tile([128, 1], f32, tag="v")
                        nc.vector.tensor_scalar(v[:], s2[:], 1.0 / 128, LN_EPS, A_OP.mult, A_OP.add)
                        sq = lnpool.tile([128, 1], f32, tag="sq")
                        nc.scalar.activation(sq[:], v[:], AF.Sqrt)
                        r = lnpool.tile([128, 1], f32, tag="r")
                        nc.vector.reciprocal(r[:], sq[:])
                        xnorm = lnpool.tile([128, 128], f16, tag="xnorm")
                        nc.vector.tensor_scalar(xnorm[:], xt[:], negm[:], r[:], A_OP.add, A_OP.mult)
                        nc.tensor.transpose(psx[:, q * 128:(q + 1) * 128], xnorm[:], ident[:])
                    nc.scalar.activation(xn[:, 3 + j * 512: 3 + (j + 1) * 512], psx[:], AF.Copy)

                # ---------- in_proj(z) + conv(in_proj(x)) + x_proj + dt ----------
                zs = fepool.tile([128, T], f16, tag="zs")
                xc0 = fepool.tile([128, T], f16, tag="xc0")   # own half
                xc1 = fepool.tile([128, T], f16, tag="xc1")
                dt_t = fepool.tile([128, T], f16, tag="dt")
                dtr = fepool.tile([DT_RANK, T], f16, tag="dtr")
                bc16 = fepool.tile([32, T], f16, tag="bc16")
                for j in range(NT):
                    w0 = 3 + j * 512
                    # z half
                    psz = mmpool.tile([128, 512], f32, tag="mm")
                    nc.tensor.matmul(psz[:], wz[:], xn[:, w0:w0 + 512])
                    nc.scalar.activation(zs[:, j * 512:(j + 1) * 512], psz[:], AF.Silu, bias=sbz[:])
                    # conv via 4 shifted matmuls per e-tile
                    for et, xc in ((0, xc0), (1, xc1)):
                        psc = mmpool.tile([128, 512], f32, tag="mm")
                        for k in range(D_CONV):
                            nc.tensor.matmul(
                                psc[:],
                                wconv[:, (et * D_CONV + k) * 128:(et * D_CONV + k + 1) * 128],
                                xn[:, w0 - 3 + k: w0 - 3 + k + 512],
                                start=(k == 0), stop=(k == D_CONV - 1))
                        nc.scalar.activation(xc[:, j * 512:(j + 1) * 512], psc[:],
                                             AF.Silu, bias=convb[:, et:et + 1])
                    # x_proj (contract both e-tiles; dt-rows and B/C-rows as
                    # separate matmuls so PSUM reads start at partition 0)
                    psdt_in = mmpool.tile([DT_RANK, 512], f32, tag="mm")
                    nc.tensor.matmul(psdt_in[:], wx[:, 0:8], xc0[:, j * 512:(j + 1) * 512],
                                     start=True, stop=False)
                    nc.tensor.matmul(psdt_in[:], wx[:, 40:48], xc1[:, j * 512:(j + 1) * 512],
                                     start=False, stop=True)
                    psbc = mmpool.tile([32, 512], f32, tag="mm")
                    nc.tensor.matmul(psbc[:], wx[:, 8:40], xc0[:, j * 512:(j + 1) * 512],
                                     start=True, stop=False)
                    nc.tensor.matmul(psbc[:], wx[:, 48:80], xc1[:, j * 512:(j + 1) * 512],
                                     start=False, stop=True)
                    nc.scalar.activation(dtr[:, j * 512:(j + 1) * 512],
                                         psdt_in[:], AF.Copy)
                    nc.scalar.activation(bc16[:, j * 512:(j + 1) * 512],
                                         psbc[:], AF.Copy)
                # dt = softplus(Wdt @ dtr + bdt) = ln(1 + exp(.)); exp pass
                # grouped, then one block-wide ln (keeps table loads rare)
                e1 = fepool.tile([128, T], f32, tag="e1")
                for j in range(NT):
                    psdt = mmpool.tile([128, 512], f32, tag="mm")
                    nc.tensor.matmul(psdt[:], wdt[:], dtr[:, j * 512:(j + 1) * 512])
                    nc.scalar.activation(e1[:, j * 512:(j + 1) * 512], psdt[:], AF.Exp, bias=bdt[:])
                nc.scalar.activation(dt_t[:], e1[:], AF.Ln, bias=1.0)

                dtx = fepool.tile([128, T], f16, tag="dtx")
                nc.vector.tensor_tensor(dtx[:], dt_t[:], xc0[:], A_OP.mult)

                # bounce B/C rows through DRAM for partition-broadcast reads
                bcd = dpool.tile([32, T], f16, tag="bcd")
                nc.sync.dma_start(out=bcd[:], in_=bc16[:])

                # ---------- selective scan over 16 states ----------
                y = fepool.tile([128, T], f16, tag="y")
                for n in range(D_STATE):
                    da = spool.tile([128, T], f32, tag="da")
                    nc.scalar.activation(da[:], dt_t[:], AF.Exp, scale=a_t[:, n:n + 1])
                    bb = spool.tile([128, T], f16, tag="bb")
                    nc.sync.dma_start(out=bb[:], in_=bcd[n:n + 1, :].partition_broadcast(128))
                    u = spool.tile([128, T], f16, tag="u")
                    nc.vector.tensor_tensor(u[:], dtx[:], bb[:], A_OP.mult)
                    h = spool.tile([128, T], f16, tag="h")
                    nc.vector.tensor_tensor_scan(h[:], da[:], u[:], carry[:, n:n + 1],
                                                 A_OP.mult, A_OP.add)
                    nc.vector.tensor_copy(carry[:, n:n + 1], h[:, T - 1:T])
                    cb = spool.tile([128, T], f16, tag="cb")
                    nc.sync.dma_start(out=cb[:], in_=bcd[16 + n:17 + n, :].partition_broadcast(128))
                    if n == 0:
                        nc.vector.tensor_tensor(y[:], h[:], cb[:], A_OP.mult)
                    else:
                        ch = spool.tile([128, T], f16, tag="ch")
                        nc.vector.tensor_tensor(ch[:], h[:], cb[:], A_OP.mult)
                        nc.vector.tensor_tensor(y[:], y[:], ch[:], A_OP.add)

                # ---------- D-skip + gate + out_proj ----------
                nc.vector.scalar_tensor_tensor(y[:], xc0[:], dsk[:], y[:], A_OP.mult, A_OP.add)
                nc.vector.tensor_tensor(y[:], y[:], zs[:], A_OP.mult)
                outsb = fepool.tile([128, T], f32, tag="outsb")
                for j in range(NT):
                    pso = mmpool.tile([128, 512], f32, tag="mm")
                    nc.tensor.matmul(pso[:], wout[:], y[:, j * 512:(j + 1) * 512])
                    nc.scalar.activation(outsb[:, j * 512:(j + 1) * 512], pso[:], AF.Copy)
                nc.sync.dma_start(out=out_d[:, t0:t0 + T], in_=outsb[:])

                prev_xn = xn
    nc.compile()
    return nc


def _get_nc():
    global _CACHED_NC
    if _CACHED_NC is None:
        _CACHED_NC = _build_nc()
    return _CACHED_NC


def _core_inputs(x_seq, p, half):
    """Per-core input dict. x_seq: (L, d_model) f32 (already flipped for bwd).
    p: dict of this direction's raw params. half: which d_inner half this
    core owns (own channels are always tile 0 / the 'own' slots)."""
    Win, convw, convb = p["Win"], p["convw"], p["convb"]
    Wx, Wdt, bdt, Alog, Dsk, Wout = p["Wx"], p["Wdt"], p["bdt"], p["Alog"], p["D"], p["Wout"]
    ln_g, ln_b = p["ln_g"], p["ln_b"]

    own = slice(half * EH, (half + 1) * EH)
    other = slice((1 - half) * EH, (2 - half) * EH)
    e_order = [own, other]

    Wg = Win * ln_g[None, :]                 # fold ln gain
    bvec = Win @ ln_b                        # fold ln bias
    Wx_in = Wg[0:D_INNER]
    bx_in = bvec[0:D_INNER]

    # conv folded weights: for tile slot s (0=own), tap k: diag(convw[:,k]) @ Win_xin
    wconvT = np.zeros((128, 2 * D_CONV * 128), np.float32)
    convb2 = np.zeros((128, 2), np.float32)
    for s, sl in enumerate(e_order):
        for k in range(D_CONV):
            Wk = convw[sl, k:k + 1] * Wx_in[sl, :]        # (128,128) = diag(w_k) @ W
            wconvT[:, (s * D_CONV + k) * 128:(s * D_CONV + k + 1) * 128] = Wk.T
        convb2[:, s] = convb[sl] + convw[sl].sum(1) * bx_in[sl]

    wzT = Wg[D_INNER + half * EH: D_INNER + (half + 1) * EH, :].T
    sbz = bvec[D_INNER + half * EH: D_INNER + (half + 1) * EH][:, None]

    wxT = np.concatenate([Wx[:, sl].T for sl in e_order], axis=1)   # (128, 80)
    wdtT = Wdt[own].T                                               # (8, 128)
    A = -np.exp(Alog[own])                                          # (128, 16)
    woutT = Wout[:, own].T                                          # (128, 128)

    return {
        "x": np.ascontiguousarray(x_seq, np.float32),
        "wconvT": wconvT.astype(np.float16),
        "wzT": np.ascontiguousarray(wzT, np.float16),
        "sbz": np.ascontiguousarray(sbz, np.float32),
        "convb": convb2,
        "wxT": np.ascontiguousarray(wxT, np.float16),
        "wdtT": np.ascontiguousarray(wdtT, np.float16),
        "bdt": np.ascontiguousarray(bdt[own][:, None], np.float32),
        "A": np.ascontiguousarray(A, np.float32),
        "Dskip": np.ascontiguousarray(Dsk[own][:, None], np.float32),
        "woutT": np.ascontiguousarray(woutT, np.float16),
        "ident": np.eye(128, dtype=np.float16),
    }


def kernel(**inputs):
    inputs = {k: np.asarray(v) for k, v in inputs.items()}
    x = inputs["x"].astype(np.float32)                       # (2,128,32,16,16)
    x_cl = x.reshape(B_SZ, D_MODEL, L)                       # (B, C, L)
    x_seq = x_cl.transpose(0, 2, 1)                          # (B, L, C)

    params = {}
    for s in ("f", "b"):
        params[s] = {
            "Win": inputs[f"Win_{s}"], "convw": inputs[f"convw_{s}"],
            "convb": inputs[f"convb_{s}"], "Wx": inputs[f"Wx_{s}"],
            "Wdt": inputs[f"Wdt_{s}"], "bdt": inputs[f"bdt_{s}"],
            "Alog": inputs[f"Alog_{s}"], "D": inputs[f"D_{s}"],
            "Wout": inputs[f"Wout_{s}"], "ln_g": inputs["ln_g"],
            "ln_b": inputs["ln_b"],
        }

    in_maps = []
    meta = []
    for b in range(B_SZ):
        for s in ("f", "b"):
            xs = x_seq[b] if s == "f" else x_seq[b, ::-1]
            for half in (0, 1):
                in_maps.append(_core_inputs(xs, params[s], half))
                meta.append((b, s))

    nc = _get_nc()
    res = run_bass_kernel_spmd(nc, in_maps, list(range(8)))

    acc = np.zeros((B_SZ, D_MODEL, L), np.float32)
    for i, (b, s) in enumerate(meta):
        o = res.results[i]["out"]                            # (d_model, L)
        if s == "b":
            o = o[:, ::-1]
        acc[b] += o
    out = x_cl + acc
    return out.reshape(x.shape).astype(np.float32)


# revision 10
# speedup vs baseline: 1.0909x; 1.0909x over previous
"""Bidirectional Mamba layer on 8 Trainium2 NeuronCores.

Sharding: core = (batch b in {0,1}) x (direction in {fwd,bwd}) x
(d_inner half in {0,1}).  Each core runs the full front-end (LN,
in_proj, conv, x_proj, dt) and the selective scan + output projection
for its 128 d_inner channels.  The host flips the sequence for the
backward direction, slices weights per core, and sums the 4 partial
(d_model, L) outputs per batch plus the residual.

One SPMD Bass graph serves all 8 cores; all per-core variation lives in
the input data (weight slices / flipped x).
"""

import math
import numpy as np

import concourse.bass as bass
import concourse.bacc as bacc
import concourse.mybir as mybir
from concourse import tile
from concourse.bass_utils import run_bass_kernel_spmd

# Problem shape (hardcoded per contract)
B_SZ = 2
D_MODEL = 128
D_STATE = 16
D_CONV = 4
EXPAND = 2
D_INNER = EXPAND * D_MODEL          # 256
DT_RANK = math.ceil(D_MODEL / 16)   # 8
LN_EPS = 1e-5
SPATIAL = (32, 16, 16)
L = 32 * 16 * 16                    # 8192
EH = 128                            # d_inner half per core
T = 2048                            # time block
NBLK = L // T
NT = T // 512                       # 512-tiles per block

f32 = mybir.dt.float32
f16 = mybir.dt.float16
A_OP = mybir.AluOpType
AF = mybir.ActivationFunctionType

_CACHED_NC = None


def _build_nc():
    nc = bacc.Bacc("TRN2", target_bir_lowering=False, debug=False, num_devices=8)

    # ---- DRAM parameters (per-core data) ----
    x_d = nc.declare_dram_parameter("x", [L, D_MODEL], f32, isOutput=False)
    wconv_d = nc.declare_dram_parameter("wconvT", [128, 2 * D_CONV * 128], f16, isOutput=False)
    wz_d = nc.declare_dram_parameter("wzT", [128, 128], f16, isOutput=False)
    sbz_d = nc.declare_dram_parameter("sbz", [128, 1], f32, isOutput=False)
    convb_d = nc.declare_dram_parameter("convb", [128, 2], f32, isOutput=False)
    wx_d = nc.declare_dram_parameter("wxT", [128, 80], f16, isOutput=False)
    wdt_d = nc.declare_dram_parameter("wdtT", [DT_RANK, 128], f16, isOutput=False)
    bdt_d = nc.declare_dram_parameter("bdt", [128, 1], f32, isOutput=False)
    a_d = nc.declare_dram_parameter("A", [128, D_STATE], f32, isOutput=False)
    dsk_d = nc.declare_dram_parameter("Dskip", [128, 1], f32, isOutput=False)
    wout_d = nc.declare_dram_parameter("woutT", [128, 128], f16, isOutput=False)
    ident_d = nc.declare_dram_parameter("ident", [128, 128], f16, isOutput=False)
    out_d = nc.declare_dram_parameter("out", [D_MODEL, L], f32, isOutput=True)

    with tile.TileContext(nc) as tc:
        with (
            tc.tile_pool(name="const", bufs=1) as cpool,
            tc.tile_pool(name="ln", bufs=3) as lnpool,
            tc.tile_pool(name="fe", bufs=2) as fepool,
            tc.tile_pool(name="scan", bufs=2) as spool,
            tc.tile_pool(name="bcast", bufs=4) as bpool,
            tc.tile_pool(name="mm", bufs=3, space="PSUM") as mmpool,
            tc.tile_pool(name="psx", bufs=2, space="PSUM") as psxpool,
            tc.tile_pool(name="dram", bufs=2, space="DRAM") as dpool,
        ):
            # ---- constants ----
            wconv = cpool.tile([128, 2 * D_CONV * 128], f16)
            wz = cpool.tile([128, 128], f16)
            sbz = cpool.tile([128, 1], f32)
            convb = cpool.tile([128, 2], f32)
            wx = cpool.tile([128, 80], f16)
            wdt = cpool.tile([DT_RANK, 128], f16)
            bdt = cpool.tile([128, 1], f32)
            a_t = cpool.tile([128, D_STATE], f32)
            dsk = cpool.tile([128, 1], f32)
            wout = cpool.tile([128, 128], f16)
            ident = cpool.tile([128, 128], f16)
            carry = cpool.tile([128, D_STATE], f32)
            for sb_t, dr in ((wconv, wconv_d), (wz, wz_d), (sbz, sbz_d),
                             (convb, convb_d), (wx, wx_d), (wdt, wdt_d),
                             (bdt, bdt_d), (a_t, a_d), (dsk, dsk_d),
                             (wout, wout_d), (ident, ident_d)):
                nc.sync.dma_start(out=sb_t[:], in_=dr[:])
            nc.vector.memset(carry[:], 0.0)

            prev_xn = None
            for blk in range(NBLK):
                t0 = blk * T
                # ---------- LN + transpose into xn (c-part, 3+T) ----------
                xn = fepool.tile([128, 3 + T], f16, tag="xn")
                if prev_xn is None:
                    nc.vector.memset(xn[:, 0:3], 0.0)
                else:
                    nc.vector.tensor_copy(xn[:, 0:3], prev_xn[:, T:T + 3])
                # LN: one block DMA, per-tile stats (Identity/Square live in
                # every act table), then ONE Ln + ONE Exp per block for
                # rsqrt = exp(-0.5*ln(v+eps)) — keeps everything on the
                # exp+ln table (same one softplus and the scan's exps use).
                xts = lnpool.tile([128, 16, 128], f32, tag="xts")
                nc.sync.dma_start(
                    out=xts[:],
                    in_=x_d[t0:t0 + T, :].rearrange("(i p) c -> p i c", p=128))
                negm16 = lnpool.tile([128, 16], f32, tag="negm16")
                v16 = lnpool.tile([128, 16], f32, tag="v16")
                for i in range(16):
                    s1 = lnpool.tile([128, 1], f32, tag="s1")
                    scr = lnpool.tile([128, 128], f32, tag="scr")
                    nc.scalar.activation(scr[:], xts[:, i, :], AF.Identity, accum_out=s1[:])
                    nc.vector.tensor_scalar(negm16[:, i:i + 1], s1[:], -1.0 / 128, None, A_OP.mult)
                    s2 = lnpool.tile([128, 1], f32, tag="s2")
                    nc.scalar.activation(scr[:], xts[:, i, :], AF.Square,
                                         bias=negm16[:, i:i + 1], accum_out=s2[:])
                    nc.vector.tensor_scalar(v16[:, i:i + 1], s2[:], 1.0 / 128, LN_EPS,
                                            A_OP.mult, A_OP.add)
                lnv16 = lnpool.tile([128, 16], f32, tag="lnv16")
                nc.scalar.activation(lnv16[:], v16[:], AF.Ln)
                r16 = lnpool.tile([128, 16], f32, tag="r16")
                nc.scalar.activation(r16[:], lnv16[:], AF.Exp, scale=-0.5)
                for j in range(NT):
                    psx = psxpool.tile([128, 512], f16, tag="psx")
                    for q in range(4):
                        i = j * 4 + q
                        xnorm = lnpool.tile([128, 128], f16, tag="xnorm")
                        nc.vector.tensor_scalar(xnorm[:], xts[:, i, :], negm16[:, i:i + 1],
                                                r16[:, i:i + 1], A_OP.add, A_OP.mult)
                        nc.tensor.transpose(psx[:, q * 128:(q + 1) * 128], xnorm[:], ident[:])
                    nc.scalar.activation(xn[:, 3 + j * 512: 3 + (j + 1) * 512], psx[:], AF.Copy)

                # ---------- in_proj(z) + conv(in_proj(x)) + x_proj + dt ----------
                zs = fepool.tile([128, T], f16, tag="zs")
                xc0 = fepool.tile([128, T], f16, tag="xc0")   # own half
                xc1 = fepool.tile([128, T], f16, tag="xc1")
                dt_t = fepool.tile([128, T], f16, tag="dt")
                dtr = fepool.tile([DT_RANK, T], f16, tag="dtr")
                bc16 = fepool.tile([32, T], f16, tag="bc16")
                for j in range(NT):
                    w0 = 3 + j * 512
                    # z half
                    psz = mmpool.tile([128, 512], f32, tag="mm")
                    nc.tensor.matmul(psz[:], wz[:], xn[:, w0:w0 + 512])
                    nc.scalar.activation(zs[:, j * 512:(j + 1) * 512], psz[:], AF.Silu, bias=sbz[:])
                    # conv via 4 shifted matmuls per e-tile
                    for et, xc in ((0, xc0), (1, xc1)):
                        psc = mmpool.tile([128, 512], f32, tag="mm")
                        for k in range(D_CONV):
                            nc.tensor.matmul(
                                psc[:],
                                wconv[:, (et * D_CONV + k) * 128:(et * D_CONV + k + 1) * 128],
                                xn[:, w0 - 3 + k: w0 - 3 + k + 512],
                                start=(k == 0), stop=(k == D_CONV - 1))
                        nc.scalar.activation(xc[:, j * 512:(j + 1) * 512], psc[:],
                                             AF.Silu, bias=convb[:, et:et + 1])
                    # x_proj (contract both e-tiles; dt-rows and B/C-rows as
                    # separate matmuls so PSUM reads start at partition 0)
                    psdt_in = mmpool.tile([DT_RANK, 512], f32, tag="mm")
                    nc.tensor.matmul(psdt_in[:], wx[:, 0:8], xc0[:, j * 512:(j + 1) * 512],
                                     start=True, stop=False)
                    nc.tensor.matmul(psdt_in[:], wx[:, 40:48], xc1[:, j * 512:(j + 1) * 512],
                                     start=False, stop=True)
                    psbc = mmpool.tile([32, 512], f32, tag="mm")
                    nc.tensor.matmul(psbc[:], wx[:, 8:40], xc0[:, j * 512:(j + 1) * 512],
                                     start=True, stop=False)
                    nc.tensor.matmul(psbc[:], wx[:, 48:80], xc1[:, j * 512:(j + 1) * 512],
                                     start=False, stop=True)
                    nc.scalar.activation(dtr[:, j * 512:(j + 1) * 512],
                                         psdt_in[:], AF.Copy)
                    nc.scalar.activation(bc16[:, j * 512:(j + 1) * 512],
                                         psbc[:], AF.Copy)
                # dt = softplus(Wdt @ dtr + bdt) = ln(1 + exp(.)); exp pass
                # grouped, then one block-wide ln (keeps table loads rare)
                e1 = fepool.tile([128, T], f32, tag="e1")
                for j in range(NT):
                    psdt = mmpool.tile([128, 512], f32, tag="mm")
                    nc.tensor.matmul(psdt[:], wdt[:], dtr[:, j * 512:(j + 1) * 512])
                    nc.scalar.activation(e1[:, j * 512:(j + 1) * 512], psdt[:], AF.Exp, bias=bdt[:])
                nc.scalar.activation(dt_t[:], e1[:], AF.Ln, bias=1.0)

                dtx = fepool.tile([128, T], f16, tag="dtx")
                nc.vector.tensor_tensor(dtx[:], dt_t[:], xc0[:], A_OP.mult)

                # bounce B/C rows through DRAM for partition-broadcast reads
                bcd = dpool.tile([32, T], f16, tag="bcd")
                nc.sync.dma_start(out=bcd[:], in_=bc16[:])

                # ---------- selective scan over 16 states ----------
                y = fepool.tile([128, T], f16, tag="y")
                for n in range(D_STATE):
                    da = spool.tile([128, T], f32, tag="da")
                    nc.scalar.activation(da[:], dt_t[:], AF.Exp, scale=a_t[:, n:n + 1])
                    bb = bpool.tile([128, T], f16, tag="bb")
                    nc.sync.dma_start(out=bb[:], in_=bcd[n:n + 1, :].partition_broadcast(128))
                    u = spool.tile([128, T], f16, tag="u")
                    nc.vector.tensor_tensor(u[:], dtx[:], bb[:], A_OP.mult)
                    h = spool.tile([128, T], f16, tag="h")
                    nc.vector.tensor_tensor_scan(h[:], da[:], u[:], carry[:, n:n + 1],
                                                 A_OP.mult, A_OP.add)
                    nc.vector.tensor_copy(carry[:, n:n + 1], h[:, T - 1:T])
                    cb = bpool.tile([128, T], f16, tag="cb")
                    nc.sync.dma_start(out=cb[:], in_=bcd[16 + n:17 + n, :].partition_broadcast(128))
                    if n == 0:
                        nc.vector.tensor_tensor(y[:], h[:], cb[:], A_OP.mult)
                    else:
                        ch = spool.tile([128, T], f16, tag="ch")
                        nc.vector.tensor_tensor(ch[:], h[:], cb[:], A_OP.mult)
                        nc.vector.tensor_tensor(y[:], y[:], ch[:], A_OP.add)

                # ---------- D-skip + gate + out_proj ----------
                nc.vector.scalar_tensor_tensor(y[:], xc0[:], dsk[:], y[:], A_OP.mult, A_OP.add)
                nc.vector.tensor_tensor(y[:], y[:], zs[:], A_OP.mult)
                outsb = fepool.tile([128, T], f32, tag="outsb")
                for j in range(NT):
                    pso = mmpool.tile([128, 512], f32, tag="mm")
                    nc.tensor.matmul(pso[:], wout[:], y[:, j * 512:(j + 1) * 512])
                    nc.scalar.activation(outsb[:, j * 512:(j + 1) * 512], pso[:], AF.Copy)
                nc.sync.dma_start(out=out_d[:, t0:t0 + T], in_=outsb[:])

                prev_xn = xn
    nc.compile()
    return nc


def _get_nc():
    global _CACHED_NC
    if _CACHED_NC is None:
        _CACHED_NC = _build_nc()
    return _CACHED_NC


def _core_inputs(x_seq, p, half):
    """Per-core input dict. x_seq: (L, d_model) f32 (already flipped for bwd).
    p: dict of this direction's raw params. half: which d_inner half this
    core owns (own channels are always tile 0 / the 'own' slots)."""
    Win, convw, convb = p["Win"], p["convw"], p["convb"]
    Wx, Wdt, bdt, Alog, Dsk, Wout = p["Wx"], p["Wdt"], p["bdt"], p["Alog"], p["D"], p["Wout"]
    ln_g, ln_b = p["ln_g"], p["ln_b"]

    own = slice(half * EH, (half + 1) * EH)
    other = slice((1 - half) * EH, (2 - half) * EH)
    e_order = [own, other]

    Wg = Win * ln_g[None, :]                 # fold ln gain
    bvec = Win @ ln_b                        # fold ln bias
    Wx_in = Wg[0:D_INNER]
    bx_in = bvec[0:D_INNER]

    # conv folded weights: for tile slot s (0=own), tap k: diag(convw[:,k]) @ Win_xin
    wconvT = np.zeros((128, 2 * D_CONV * 128), np.float32)
    convb2 = np.zeros((128, 2), np.float32)
    for s, sl in enumerate(e_order):
        for k in range(D_CONV):
            Wk = convw[sl, k:k + 1] * Wx_in[sl, :]        # (128,128) = diag(w_k) @ W
            wconvT[:, (s * D_CONV + k) * 128:(s * D_CONV + k + 1) * 128] = Wk.T
        convb2[:, s] = convb[sl] + convw[sl].sum(1) * bx_in[sl]

    wzT = Wg[D_INNER + half * EH: D_INNER + (half + 1) * EH, :].T
    sbz = bvec[D_INNER + half * EH: D_INNER + (half + 1) * EH][:, None]

    wxT = np.concatenate([Wx[:, sl].T for sl in e_order], axis=1)   # (128, 80)
    wdtT = Wdt[own].T                                               # (8, 128)
    A = -np.exp(Alog[own])                                          # (128, 16)
    woutT = Wout[:, own].T                                          # (128, 128)

    return {
        "x": np.ascontiguousarray(x_seq, np.float32),
        "wconvT": wconvT.astype(np.float16),
        "wzT": np.ascontiguousarray(wzT, np.float16),
        "sbz": np.ascontiguousarray(sbz, np.float32),
        "convb": convb2,
        "wxT": np.ascontiguousarray(wxT, np.float16),
        "wdtT": np.ascontiguousarray(wdtT, np.float16),
        "bdt": np.ascontiguousarray(bdt[own][:, None], np.float32),
        "A": np.ascontiguousarray(A, np.float32),
        "Dskip": np.ascontiguousarray(Dsk[own][:, None], np.float32),
        "woutT": np.ascontiguousarray(woutT, np.float16),
        "ident": np.eye(128, dtype=np.float16),
    }


def kernel(**inputs):
    inputs = {k: np.asarray(v) for k, v in inputs.items()}
    x = inputs["x"].astype(np.float32)                       # (2,128,32,16,16)
    x_cl = x.reshape(B_SZ, D_MODEL, L)                       # (B, C, L)
    x_seq = x_cl.transpose(0, 2, 1)                          # (B, L, C)

    params = {}
    for s in ("f", "b"):
        params[s] = {
            "Win": inputs[f"Win_{s}"], "convw": inputs[f"convw_{s}"],
            "convb": inputs[f"convb_{s}"], "Wx": inputs[f"Wx_{s}"],
            "Wdt": inputs[f"Wdt_{s}"], "bdt": inputs[f"bdt_{s}"],
            "Alog": inputs[f"Alog_{s}"], "D": inputs[f"D_{s}"],
            "Wout": inputs[f"Wout_{s}"], "ln_g": inputs["ln_g"],
            "ln_b": inputs["ln_b"],
        }

    in_maps = []
    meta = []
    for b in range(B_SZ):
        for s in ("f", "b"):
            xs = x_seq[b] if s == "f" else x_seq[b, ::-1]
            for half in (0, 1):
                in_maps.append(_core_inputs(xs, params[s], half))
                meta.append((b, s))

    nc = _get_nc()
    res = run_bass_kernel_spmd(nc, in_maps, list(range(8)))

    acc = np.zeros((B_SZ, D_MODEL, L), np.float32)
    for i, (b, s) in enumerate(meta):
        o = res.results[i]["out"]                            # (d_model, L)
        if s == "b":
            o = o[:, ::-1]
        acc[b] += o
    out = x_cl + acc
    return out.reshape(x.shape).astype(np.float32)


# revision 13
# speedup vs baseline: 1.1351x; 1.0406x over previous
"""Bidirectional Mamba layer on 8 Trainium2 NeuronCores.

Sharding: core = (batch b in {0,1}) x (direction in {fwd,bwd}) x
(d_inner half in {0,1}).  Each core runs the full front-end (LN,
in_proj, conv, x_proj, dt) and the selective scan + output projection
for its 128 d_inner channels.  The host flips the sequence for the
backward direction, slices weights per core, and sums the 4 partial
(d_model, L) outputs per batch plus the residual.

One SPMD Bass graph serves all 8 cores; all per-core variation lives in
the input data (weight slices / flipped x).
"""

import math
import numpy as np

import concourse.bass as bass
import concourse.bacc as bacc
import concourse.mybir as mybir
from concourse import tile
from concourse.bass_utils import run_bass_kernel_spmd

# Problem shape (hardcoded per contract)
B_SZ = 2
D_MODEL = 128
D_STATE = 16
D_CONV = 4
EXPAND = 2
D_INNER = EXPAND * D_MODEL          # 256
DT_RANK = math.ceil(D_MODEL / 16)   # 8
LN_EPS = 1e-5
SPATIAL = (32, 16, 16)
L = 32 * 16 * 16                    # 8192
EH = 128                            # d_inner half per core
T = 2048                            # time block
NBLK = L // T
NT = T // 512                       # 512-tiles per block

f32 = mybir.dt.float32
f16 = mybir.dt.float16
A_OP = mybir.AluOpType
AF = mybir.ActivationFunctionType

_CACHED_NC = None


def _build_nc():
    nc = bacc.Bacc("TRN2", target_bir_lowering=False, debug=False, num_devices=8)

    # ---- DRAM parameters (per-core data) ----
    x_d = nc.declare_dram_parameter("x", [L, D_MODEL], f32, isOutput=False)
    wconv_d = nc.declare_dram_parameter("wconvT", [128, 2 * D_CONV * 128], f16, isOutput=False)
    wz_d = nc.declare_dram_parameter("wzT", [128, 128], f16, isOutput=False)
    sbz_d = nc.declare_dram_parameter("sbz", [128, 1], f32, isOutput=False)
    convb_d = nc.declare_dram_parameter("convb", [128, 2], f32, isOutput=False)
    wx_d = nc.declare_dram_parameter("wxT", [128, 80], f16, isOutput=False)
    wdt_d = nc.declare_dram_parameter("wdtT", [DT_RANK, 128], f16, isOutput=False)
    bdt_d = nc.declare_dram_parameter("bdt", [128, 1], f32, isOutput=False)
    a_d = nc.declare_dram_parameter("A", [128, D_STATE], f32, isOutput=False)
    dsk_d = nc.declare_dram_parameter("Dskip", [128, 1], f32, isOutput=False)
    wout_d = nc.declare_dram_parameter("woutT", [128, 128], f16, isOutput=False)
    ident_d = nc.declare_dram_parameter("ident", [128, 128], f16, isOutput=False)
    out_d = nc.declare_dram_parameter("out", [D_MODEL, L], f32, isOutput=True)

    with tile.TileContext(nc) as tc:
        with (
            tc.tile_pool(name="const", bufs=1) as cpool,
            tc.tile_pool(name="ln", bufs=3) as lnpool,
            tc.tile_pool(name="fe", bufs=2) as fepool,
            tc.tile_pool(name="scan", bufs=2) as spool,
            tc.tile_pool(name="bcast", bufs=4) as bpool,
            tc.tile_pool(name="scr1", bufs=1) as scrpool,
            tc.tile_pool(name="mm", bufs=3, space="PSUM") as mmpool,
            tc.tile_pool(name="psx", bufs=2, space="PSUM") as psxpool,
            tc.tile_pool(name="dram", bufs=2, space="DRAM") as dpool,
        ):
            # ---- constants ----
            wconv = cpool.tile([128, 2 * D_CONV * 128], f16)
            wz = cpool.tile([128, 128], f16)
            sbz = cpool.tile([128, 1], f32)
            convb = cpool.tile([128, 2], f32)
            wx = cpool.tile([128, 80], f16)
            wdt = cpool.tile([DT_RANK, 128], f16)
            bdt = cpool.tile([128, 1], f32)
            a_t = cpool.tile([128, D_STATE], f32)
            dsk = cpool.tile([128, 1], f32)
            wout = cpool.tile([128, 128], f16)
            ident = cpool.tile([128, 128], f16)
            carry = cpool.tile([128, D_STATE], f32)
            for sb_t, dr in ((wconv, wconv_d), (wz, wz_d), (sbz, sbz_d),
                             (convb, convb_d), (wx, wx_d), (wdt, wdt_d),
                             (bdt, bdt_d), (a_t, a_d), (dsk, dsk_d),
                             (wout, wout_d), (ident, ident_d)):
                nc.sync.dma_start(out=sb_t[:], in_=dr[:])
            nc.vector.memset(carry[:], 0.0)

            prev_xn = None
            for blk in range(NBLK):
                t0 = blk * T
                # ---------- LN + transpose into xn (c-part, 3+T) ----------
                xn = fepool.tile([128, 3 + T], f16, tag="xn")
                if prev_xn is None:
                    nc.vector.memset(xn[:, 0:3], 0.0)
                else:
                    nc.vector.tensor_copy(xn[:, 0:3], prev_xn[:, T:T + 3])
                # LN: one block DMA, per-tile stats (Identity/Square live in
                # every act table), then ONE Ln + ONE Exp per block for
                # rsqrt = exp(-0.5*ln(v+eps)) — keeps everything on the
                # exp+ln table (same one softplus and the scan's exps use).
                xts = fepool.tile([128, 16, 128], f32, tag="xts")
                nc.sync.dma_start(
                    out=xts[:],
                    in_=x_d[t0:t0 + T, :].rearrange("(i p) c -> p i c", p=128))
                # stats via DVE reduces (no per-tile ScalarE round-trips):
                # m = mean(x), v = mean(x^2) - m^2, r = exp(-0.5*ln(v+eps))
                xsq = scrpool.tile([128, 16, 128], f32, tag="xsq")
                nc.vector.tensor_tensor(xsq[:], xts[:], xts[:], A_OP.mult)
                s1r = lnpool.tile([128, 16], f32, tag="s1r")
                nc.vector.tensor_reduce(s1r[:], xts[:], mybir.AxisListType.X, A_OP.add)
                s2r = lnpool.tile([128, 16], f32, tag="s2r")
                nc.vector.tensor_reduce(s2r[:], xsq[:], mybir.AxisListType.X, A_OP.add)
                negm16 = lnpool.tile([128, 16], f32, tag="negm16")
                nc.vector.tensor_scalar(negm16[:], s1r[:], -1.0 / 128, None, A_OP.mult)
                m2 = lnpool.tile([128, 16], f32, tag="m2")
                nc.scalar.activation(m2[:], s1r[:], AF.Square, scale=1.0 / 128)
                v16 = lnpool.tile([128, 16], f32, tag="v16")
                nc.vector.tensor_scalar(v16[:], s2r[:], 1.0 / 128, LN_EPS, A_OP.mult, A_OP.add)
                nc.vector.tensor_tensor(v16[:], v16[:], m2[:], A_OP.subtract)
                lnv16 = lnpool.tile([128, 16], f32, tag="lnv16")
                nc.scalar.activation(lnv16[:], v16[:], AF.Ln)
                r16 = lnpool.tile([128, 16], f32, tag="r16")
                nc.scalar.activation(r16[:], lnv16[:], AF.Exp, scale=-0.5)
                for j in range(NT):
                    psx = psxpool.tile([128, 512], f16, tag="psx")
                    for q in range(4):
                        i = j * 4 + q
                        xnorm = lnpool.tile([128, 128], f16, tag="xnorm")
                        nc.vector.tensor_scalar(xnorm[:], xts[:, i, :], negm16[:, i:i + 1],
                                                r16[:, i:i + 1], A_OP.add, A_OP.mult)
                        nc.tensor.transpose(psx[:, q * 128:(q + 1) * 128], xnorm[:], ident[:])
                    nc.scalar.activation(xn[:, 3 + j * 512: 3 + (j + 1) * 512], psx[:], AF.Copy)

                # ---------- in_proj(z) + conv(in_proj(x)) + x_proj + dt ----------
                zs = fepool.tile([128, T], f16, tag="zs")
                xc0 = fepool.tile([128, T], f16, tag="xc0")   # own half
                xc1 = fepool.tile([128, T], f16, tag="xc1")
                dt_t = fepool.tile([128, T], f16, tag="dt")
                dtr = fepool.tile([DT_RANK, T], f16, tag="dtr")
                bc16 = fepool.tile([32, T], f16, tag="bc16")
                for j in range(NT):
                    w0 = 3 + j * 512
                    # z half
                    psz = mmpool.tile([128, 512], f32, tag="mm")
                    nc.tensor.matmul(psz[:], wz[:], xn[:, w0:w0 + 512])
                    nc.scalar.activation(zs[:, j * 512:(j + 1) * 512], psz[:], AF.Silu, bias=sbz[:])
                    # conv via 4 shifted matmuls per e-tile
                    for et, xc in ((0, xc0), (1, xc1)):
                        psc = mmpool.tile([128, 512], f32, tag="mm")
                        for k in range(D_CONV):
                            nc.tensor.matmul(
                                psc[:],
                                wconv[:, (et * D_CONV + k) * 128:(et * D_CONV + k + 1) * 128],
                                xn[:, w0 - 3 + k: w0 - 3 + k + 512],
                                start=(k == 0), stop=(k == D_CONV - 1))
                        nc.scalar.activation(xc[:, j * 512:(j + 1) * 512], psc[:],
                                             AF.Silu, bias=convb[:, et:et + 1])
                    # x_proj (contract both e-tiles; dt-rows and B/C-rows as
                    # separate matmuls so PSUM reads start at partition 0)
                    psdt_in = mmpool.tile([DT_RANK, 512], f32, tag="mm")
                    nc.tensor.matmul(psdt_in[:], wx[:, 0:8], xc0[:, j * 512:(j + 1) * 512],
                                     start=True, stop=False)
                    nc.tensor.matmul(psdt_in[:], wx[:, 40:48], xc1[:, j * 512:(j + 1) * 512],
                                     start=False, stop=True)
                    psbc = mmpool.tile([32, 512], f32, tag="mm")
                    nc.tensor.matmul(psbc[:], wx[:, 8:40], xc0[:, j * 512:(j + 1) * 512],
                                     start=True, stop=False)
                    nc.tensor.matmul(psbc[:], wx[:, 48:80], xc1[:, j * 512:(j + 1) * 512],
                                     start=False, stop=True)
                    nc.scalar.activation(dtr[:, j * 512:(j + 1) * 512],
                                         psdt_in[:], AF.Copy)
                    nc.scalar.activation(bc16[:, j * 512:(j + 1) * 512],
                                         psbc[:], AF.Copy)
                # dt = softplus(Wdt @ dtr + bdt) = ln(1 + exp(.)); exp pass
                # grouped, then one block-wide ln (keeps table loads rare)
                e1 = fepool.tile([128, T], f16, tag="e1")
                for j in range(NT):
                    psdt = mmpool.tile([128, 512], f32, tag="mm")
                    nc.tensor.matmul(psdt[:], wdt[:], dtr[:, j * 512:(j + 1) * 512])
                    nc.scalar.activation(e1[:, j * 512:(j + 1) * 512], psdt[:], AF.Exp, bias=bdt[:])
                nc.scalar.activation(dt_t[:], e1[:], AF.Ln, bias=1.0)

                dtx = fepool.tile([128, T], f16, tag="dtx")
                nc.vector.tensor_tensor(dtx[:], dt_t[:], xc0[:], A_OP.mult)

                # bounce B/C rows through DRAM for partition-broadcast reads
                bcd = dpool.tile([32, T], f16, tag="bcd")
                nc.sync.dma_start(out=bcd[:], in_=bc16[:])

                # ---------- selective scan over 16 states ----------
                y = fepool.tile([128, T], f16, tag="y")
                for n in range(D_STATE):
                    da = spool.tile([128, T], f32, tag="da")
                    nc.scalar.activation(da[:], dt_t[:], AF.Exp, scale=a_t[:, n:n + 1])
                    bb = bpool.tile([128, T], f16, tag="bb")
                    nc.sync.dma_start(out=bb[:], in_=bcd[n:n + 1, :].partition_broadcast(128))
                    u = spool.tile([128, T], f16, tag="u")
                    nc.vector.tensor_tensor(u[:], dtx[:], bb[:], A_OP.mult)
                    h = spool.tile([128, T], f16, tag="h")
                    nc.vector.tensor_tensor_scan(h[:], da[:], u[:], carry[:, n:n + 1],
                                                 A_OP.mult, A_OP.add)
                    nc.vector.tensor_copy(carry[:, n:n + 1], h[:, T - 1:T])
                    cb = bpool.tile([128, T], f16, tag="cb")
                    nc.sync.dma_start(out=cb[:], in_=bcd[16 + n:17 + n, :].partition_broadcast(128))
                    if n == 0:
                        nc.vector.tensor_tensor(y[:], h[:], cb[:], A_OP.mult)
                    else:
                        ch = spool.tile([128, T], f16, tag="ch")
                        nc.vector.tensor_tensor(ch[:], h[:], cb[:], A_OP.mult)
                        nc.vector.tensor_tensor(y[:], y[:], ch[:], A_OP.add)

                # ---------- D-skip + gate + out_proj ----------
                nc.vector.scalar_tensor_tensor(y[:], xc0[:], dsk[:], y[:], A_OP.mult, A_OP.add)
                nc.vector.tensor_tensor(y[:], y[:], zs[:], A_OP.mult)
                outsb = fepool.tile([128, T], f32, tag="outsb")
                for j in range(NT):
                    pso = mmpool.tile([128, 512], f32, tag="mm")
                    nc.tensor.matmul(pso[:], wout[:], y[:, j * 512:(j + 1) * 512])
                    nc.scalar.activation(outsb[:, j * 512:(j + 1) * 512], pso[:], AF.Copy)
                nc.sync.dma_start(out=out_d[:, t0:t0 + T], in_=outsb[:])

                prev_xn = xn
    nc.compile()
    return nc


def _get_nc():
    global _CACHED_NC
    if _CACHED_NC is None:
        _CACHED_NC = _build_nc()
    return _CACHED_NC


def _core_inputs(x_seq, p, half):
    """Per-core input dict. x_seq: (L, d_model) f32 (already flipped for bwd).
    p: dict of this direction's raw params. half: which d_inner half this
    core owns (own channels are always tile 0 / the 'own' slots)."""
    Win, convw, convb = p["Win"], p["convw"], p["convb"]
    Wx, Wdt, bdt, Alog, Dsk, Wout = p["Wx"], p["Wdt"], p["bdt"], p["Alog"], p["D"], p["Wout"]
    ln_g, ln_b = p["ln_g"], p["ln_b"]

    own = slice(half * EH, (half + 1) * EH)
    other = slice((1 - half) * EH, (2 - half) * EH)
    e_order = [own, other]

    Wg = Win * ln_g[None, :]                 # fold ln gain
    bvec = Win @ ln_b                        # fold ln bias
    Wx_in = Wg[0:D_INNER]
    bx_in = bvec[0:D_INNER]

    # conv folded weights: for tile slot s (0=own), tap k: diag(convw[:,k]) @ Win_xin
    wconvT = np.zeros((128, 2 * D_CONV * 128), np.float32)
    convb2 = np.zeros((128, 2), np.float32)
    for s, sl in enumerate(e_order):
        for k in range(D_CONV):
            Wk = convw[sl, k:k + 1] * Wx_in[sl, :]        # (128,128) = diag(w_k) @ W
            wconvT[:, (s * D_CONV + k) * 128:(s * D_CONV + k + 1) * 128] = Wk.T
        convb2[:, s] = convb[sl] + convw[sl].sum(1) * bx_in[sl]

    wzT = Wg[D_INNER + half * EH: D_INNER + (half + 1) * EH, :].T
    sbz = bvec[D_INNER + half * EH: D_INNER + (half + 1) * EH][:, None]

    wxT = np.concatenate([Wx[:, sl].T for sl in e_order], axis=1)   # (128, 80)
    wdtT = Wdt[own].T                                               # (8, 128)
    A = -np.exp(Alog[own])                                          # (128, 16)
    woutT = Wout[:, own].T                                          # (128, 128)

    return {
        "x": np.ascontiguousarray(x_seq, np.float32),
        "wconvT": wconvT.astype(np.float16),
        "wzT": np.ascontiguousarray(wzT, np.float16),
        "sbz": np.ascontiguousarray(sbz, np.float32),
        "convb": convb2,
        "wxT": np.ascontiguousarray(wxT, np.float16),
        "wdtT": np.ascontiguousarray(wdtT, np.float16),
        "bdt": np.ascontiguousarray(bdt[own][:, None], np.float32),
        "A": np.ascontiguousarray(A, np.float32),
        "Dskip": np.ascontiguousarray(Dsk[own][:, None], np.float32),
        "woutT": np.ascontiguousarray(woutT, np.float16),
        "ident": np.eye(128, dtype=np.float16),
    }


def kernel(**inputs):
    inputs = {k: np.asarray(v) for k, v in inputs.items()}
    x = inputs["x"].astype(np.float32)                       # (2,128,32,16,16)
    x_cl = x.reshape(B_SZ, D_MODEL, L)                       # (B, C, L)
    x_seq = x_cl.transpose(0, 2, 1)                          # (B, L, C)

    params = {}
    for s in ("f", "b"):
        params[s] = {
            "Win": inputs[f"Win_{s}"], "convw": inputs[f"convw_{s}"],
            "convb": inputs[f"convb_{s}"], "Wx": inputs[f"Wx_{s}"],
            "Wdt": inputs[f"Wdt_{s}"], "bdt": inputs[f"bdt_{s}"],
            "Alog": inputs[f"Alog_{s}"], "D": inputs[f"D_{s}"],
            "Wout": inputs[f"Wout_{s}"], "ln_g": inputs["ln_g"],
            "ln_b": inputs["ln_b"],
        }

    in_maps = []
    meta = []
    for b in range(B_SZ):
        for s in ("f", "b"):
            xs = x_seq[b] if s == "f" else x_seq[b, ::-1]
            for half in (0, 1):
                in_maps.append(_core_inputs(xs, params[s], half))
                meta.append((b, s))

    nc = _get_nc()
    res = run_bass_kernel_spmd(nc, in_maps, list(range(8)))

    acc = np.zeros((B_SZ, D_MODEL, L), np.float32)
    for i, (b, s) in enumerate(meta):
        o = res.results[i]["out"]                            # (d_model, L)
        if s == "b":
            o = o[:, ::-1]
        acc[b] += o
    out = x_cl + acc
    return out.reshape(x.shape).astype(np.float32)


# revision 17
# speedup vs baseline: 1.2734x; 1.1219x over previous
"""Bidirectional Mamba layer on 8 Trainium2 NeuronCores.

Sharding: core = (batch b in {0,1}) x (direction in {fwd,bwd}) x
(d_inner half in {0,1}).  Each core runs the full front-end (LN,
in_proj, conv, x_proj, dt) and the selective scan + output projection
for its 128 d_inner channels.  The host flips the sequence for the
backward direction, slices weights per core, and sums the 4 partial
(d_model, L) outputs per batch plus the residual.

One SPMD Bass graph serves all 8 cores; all per-core variation lives in
the input data (weight slices / flipped x).
"""

import math
import numpy as np

import concourse.bass as bass
import concourse.bacc as bacc
import concourse.mybir as mybir
from concourse import tile
from concourse.bass_utils import run_bass_kernel_spmd

# Problem shape (hardcoded per contract)
B_SZ = 2
D_MODEL = 128
D_STATE = 16
D_CONV = 4
EXPAND = 2
D_INNER = EXPAND * D_MODEL          # 256
DT_RANK = math.ceil(D_MODEL / 16)   # 8
LN_EPS = 1e-5
SPATIAL = (32, 16, 16)
L = 32 * 16 * 16                    # 8192
EH = 128                            # d_inner half per core
T = 2048                            # time block
NBLK = L // T
NT = T // 512                       # 512-tiles per block

f32 = mybir.dt.float32
f16 = mybir.dt.float16
A_OP = mybir.AluOpType
AF = mybir.ActivationFunctionType

_CACHED_NC = None


def _build_nc():
    nc = bacc.Bacc("TRN2", target_bir_lowering=False, debug=False, num_devices=8)

    # ---- DRAM parameters (per-core data) ----
    x_d = nc.declare_dram_parameter("x", [L, D_MODEL], f32, isOutput=False)
    wconv_d = nc.declare_dram_parameter("wconvT", [128, 2 * D_CONV * 128], f16, isOutput=False)
    wz_d = nc.declare_dram_parameter("wzT", [128, 128], f16, isOutput=False)
    sbz_d = nc.declare_dram_parameter("sbz", [128, 1], f32, isOutput=False)
    convb_d = nc.declare_dram_parameter("convb", [128, 2], f32, isOutput=False)
    wx_d = nc.declare_dram_parameter("wxT", [128, 80], f16, isOutput=False)
    wdt_d = nc.declare_dram_parameter("wdtT", [DT_RANK, 128], f16, isOutput=False)
    bdt_d = nc.declare_dram_parameter("bdt", [128, 1], f32, isOutput=False)
    a_d = nc.declare_dram_parameter("A", [128, D_STATE], f32, isOutput=False)
    dsk_d = nc.declare_dram_parameter("Dskip", [128, 1], f32, isOutput=False)
    wout_d = nc.declare_dram_parameter("woutT", [128, 128], f16, isOutput=False)
    ident_d = nc.declare_dram_parameter("ident", [128, 128], f16, isOutput=False)
    out_d = nc.declare_dram_parameter("out", [D_MODEL, L], f32, isOutput=True)

    with tile.TileContext(nc) as tc:
        with (
            tc.tile_pool(name="const", bufs=1) as cpool,
            tc.tile_pool(name="ln", bufs=3) as lnpool,
            tc.tile_pool(name="fe", bufs=2) as fepool,
            tc.tile_pool(name="scan", bufs=2) as spool,
            tc.tile_pool(name="bcast", bufs=4) as bpool,
            tc.tile_pool(name="scr1", bufs=1) as scrpool,
            tc.tile_pool(name="mm", bufs=3, space="PSUM") as mmpool,
            tc.tile_pool(name="psx", bufs=2, space="PSUM") as psxpool,
            tc.tile_pool(name="dram", bufs=2, space="DRAM") as dpool,
        ):
            # ---- constants ----
            wconv = cpool.tile([128, 2 * D_CONV * 128], f16)
            wz = cpool.tile([128, 128], f16)
            sbz = cpool.tile([128, 1], f32)
            convb = cpool.tile([128, 2], f32)
            wx = cpool.tile([128, 80], f16)
            wdt = cpool.tile([DT_RANK, 128], f16)
            bdt = cpool.tile([128, 1], f32)
            a_t = cpool.tile([128, D_STATE], f32)
            dsk = cpool.tile([128, 1], f32)
            wout = cpool.tile([128, 128], f16)
            ident = cpool.tile([128, 128], f16)
            carry = cpool.tile([128, D_STATE], f32)
            for sb_t, dr in ((wconv, wconv_d), (wz, wz_d), (sbz, sbz_d),
                             (convb, convb_d), (wx, wx_d), (wdt, wdt_d),
                             (bdt, bdt_d), (a_t, a_d), (dsk, dsk_d),
                             (wout, wout_d), (ident, ident_d)):
                nc.sync.dma_start(out=sb_t[:], in_=dr[:])
            nc.vector.memset(carry[:], 0.0)

            prev_xn_box = [None]

            def frontend(blk):
                prev_xn = prev_xn_box[0]
                t0 = blk * T
                # ---------- LN + transpose into xn (c-part, 3+T) ----------
                xn = fepool.tile([128, 3 + T], f16, tag="xn")
                if prev_xn is None:
                    nc.vector.memset(xn[:, 0:3], 0.0)
                else:
                    nc.vector.tensor_copy(xn[:, 0:3], prev_xn[:, T:T + 3])
                # LN: one block DMA, per-tile stats (Identity/Square live in
                # every act table), then ONE Ln + ONE Exp per block for
                # rsqrt = exp(-0.5*ln(v+eps)) — keeps everything on the
                # exp+ln table (same one softplus and the scan's exps use).
                xts = fepool.tile([128, 16, 128], f32, tag="xts")
                nc.sync.dma_start(
                    out=xts[:],
                    in_=x_d[t0:t0 + T, :].rearrange("(i p) c -> p i c", p=128))
                # stats via DVE reduces (no per-tile ScalarE round-trips):
                # m = mean(x), v = mean(x^2) - m^2, r = exp(-0.5*ln(v+eps))
                xsq = scrpool.tile([128, 16, 128], f32, tag="xsq")
                nc.scalar.activation(xsq[:], xts[:], AF.Square)
                s1r = lnpool.tile([128, 16], f32, tag="s1r")
                nc.vector.tensor_reduce(s1r[:], xts[:], mybir.AxisListType.X, A_OP.add)
                s2r = lnpool.tile([128, 16], f32, tag="s2r")
                nc.vector.tensor_reduce(s2r[:], xsq[:], mybir.AxisListType.X, A_OP.add)
                negm16 = lnpool.tile([128, 16], f32, tag="negm16")
                nc.vector.tensor_scalar(negm16[:], s1r[:], -1.0 / 128, None, A_OP.mult)
                m2 = lnpool.tile([128, 16], f32, tag="m2")
                nc.scalar.activation(m2[:], s1r[:], AF.Square, scale=1.0 / 128)
                v16 = lnpool.tile([128, 16], f32, tag="v16")
                nc.vector.tensor_scalar(v16[:], s2r[:], 1.0 / 128, LN_EPS, A_OP.mult, A_OP.add)
                nc.vector.tensor_tensor(v16[:], v16[:], m2[:], A_OP.subtract)
                lnv16 = lnpool.tile([128, 16], f32, tag="lnv16")
                nc.scalar.activation(lnv16[:], v16[:], AF.Ln)
                r16 = lnpool.tile([128, 16], f32, tag="r16")
                nc.scalar.activation(r16[:], lnv16[:], AF.Exp, scale=-0.5)
                for j in range(NT):
                    psx = psxpool.tile([128, 512], f16, tag="psx")
                    for q in range(4):
                        i = j * 4 + q
                        xnorm = lnpool.tile([128, 128], f16, tag="xnorm")
                        nc.vector.tensor_scalar(xnorm[:], xts[:, i, :], negm16[:, i:i + 1],
                                                r16[:, i:i + 1], A_OP.add, A_OP.mult)
                        nc.tensor.transpose(psx[:, q * 128:(q + 1) * 128], xnorm[:], ident[:])
                    nc.scalar.activation(xn[:, 3 + j * 512: 3 + (j + 1) * 512], psx[:], AF.Copy)

                # ---------- in_proj(z) + conv(in_proj(x)) + x_proj + dt ----------
                zs = fepool.tile([128, T], f16, tag="zs")
                xc0 = fepool.tile([128, T], f16, tag="xc0")   # own half
                xc1 = fepool.tile([128, T], f16, tag="xc1")
                dt_t = fepool.tile([128, T], f16, tag="dt")
                dtr = fepool.tile([DT_RANK, T], f16, tag="dtr")
                bc16 = fepool.tile([32, T], f16, tag="bc16")
                for j in range(NT):
                    w0 = 3 + j * 512
                    # z half
                    psz = mmpool.tile([128, 512], f32, tag="mm")
                    nc.tensor.matmul(psz[:], wz[:], xn[:, w0:w0 + 512])
                    nc.scalar.activation(zs[:, j * 512:(j + 1) * 512], psz[:], AF.Silu, bias=sbz[:])
                    # conv via 4 shifted matmuls per e-tile
                    for et, xc in ((0, xc0), (1, xc1)):
                        psc = mmpool.tile([128, 512], f32, tag="mm")
                        for k in range(D_CONV):
                            nc.tensor.matmul(
                                psc[:],
                                wconv[:, (et * D_CONV + k) * 128:(et * D_CONV + k + 1) * 128],
                                xn[:, w0 - 3 + k: w0 - 3 + k + 512],
                                start=(k == 0), stop=(k == D_CONV - 1))
                        nc.scalar.activation(xc[:, j * 512:(j + 1) * 512], psc[:],
                                             AF.Silu, bias=convb[:, et:et + 1])
                    # x_proj (contract both e-tiles; dt-rows and B/C-rows as
                    # separate matmuls so PSUM reads start at partition 0)
                    psdt_in = mmpool.tile([DT_RANK, 512], f32, tag="mm")
                    nc.tensor.matmul(psdt_in[:], wx[:, 0:8], xc0[:, j * 512:(j + 1) * 512],
                                     start=True, stop=False)
                    nc.tensor.matmul(psdt_in[:], wx[:, 40:48], xc1[:, j * 512:(j + 1) * 512],
                                     start=False, stop=True)
                    psbc = mmpool.tile([32, 512], f32, tag="mm")
                    nc.tensor.matmul(psbc[:], wx[:, 8:40], xc0[:, j * 512:(j + 1) * 512],
                                     start=True, stop=False)
                    nc.tensor.matmul(psbc[:], wx[:, 48:80], xc1[:, j * 512:(j + 1) * 512],
                                     start=False, stop=True)
                    nc.scalar.activation(dtr[:, j * 512:(j + 1) * 512],
                                         psdt_in[:], AF.Copy)
                    nc.scalar.activation(bc16[:, j * 512:(j + 1) * 512],
                                         psbc[:], AF.Copy)
                # dt = softplus(Wdt @ dtr + bdt) = ln(1 + exp(.)); exp pass
                # grouped, then one block-wide ln (keeps table loads rare)
                e1 = fepool.tile([128, T], f16, tag="e1")
                for j in range(NT):
                    psdt = mmpool.tile([128, 512], f32, tag="mm")
                    nc.tensor.matmul(psdt[:], wdt[:], dtr[:, j * 512:(j + 1) * 512])
                    nc.scalar.activation(e1[:, j * 512:(j + 1) * 512], psdt[:], AF.Exp, bias=bdt[:])
                nc.scalar.activation(dt_t[:], e1[:], AF.Ln, bias=1.0)

                dtx = fepool.tile([128, T], f16, tag="dtx")
                nc.vector.tensor_tensor(dtx[:], dt_t[:], xc0[:], A_OP.mult)

                # bounce B/C rows through DRAM for partition-broadcast reads
                bcd = dpool.tile([32, T], f16, tag="bcd")
                nc.sync.dma_start(out=bcd[:], in_=bc16[:])
                prev_xn_box[0] = xn
                return dict(t0=t0, zs=zs, xc0=xc0, dt_t=dt_t, dtx=dtx, bcd=bcd)

            def scan_block(fe):
                t0, zs, xc0, dt_t, dtx, bcd = (fe["t0"], fe["zs"], fe["xc0"],
                                               fe["dt_t"], fe["dtx"], fe["bcd"])
                # ---------- selective scan over 16 states ----------
                y = fepool.tile([128, T], f16, tag="y")
                for n in range(D_STATE):
                    da = spool.tile([128, T], f32, tag="da")
                    nc.scalar.activation(da[:], dt_t[:], AF.Exp, scale=a_t[:, n:n + 1])
                    bb = bpool.tile([128, T], f16, tag="bb")
                    nc.sync.dma_start(out=bb[:], in_=bcd[n:n + 1, :].partition_broadcast(128))
                    u = spool.tile([128, T], f16, tag="u")
                    nc.vector.tensor_tensor(u[:], dtx[:], bb[:], A_OP.mult)
                    h = spool.tile([128, T], f16, tag="h")
                    nc.vector.tensor_tensor_scan(h[:], da[:], u[:], carry[:, n:n + 1],
                                                 A_OP.mult, A_OP.add)
                    nc.scalar.activation(carry[:, n:n + 1], h[:, T - 1:T], AF.Copy)
                    cb = bpool.tile([128, T], f16, tag="cb")
                    nc.sync.dma_start(out=cb[:], in_=bcd[16 + n:17 + n, :].partition_broadcast(128))
                    if n == 0:
                        nc.vector.tensor_tensor(y[:], h[:], cb[:], A_OP.mult)
                    else:
                        ch = spool.tile([128, T], f16, tag="ch")
                        nc.vector.tensor_tensor(ch[:], h[:], cb[:], A_OP.mult)
                        nc.vector.tensor_tensor(y[:], y[:], ch[:], A_OP.add)

                # ---------- D-skip + gate + out_proj ----------
                nc.vector.scalar_tensor_tensor(y[:], xc0[:], dsk[:], y[:], A_OP.mult, A_OP.add)
                nc.vector.tensor_tensor(y[:], y[:], zs[:], A_OP.mult)
                outsb = fepool.tile([128, T], f32, tag="outsb")
                for j in range(NT):
                    pso = mmpool.tile([128, 512], f32, tag="mm")
                    nc.tensor.matmul(pso[:], wout[:], y[:, j * 512:(j + 1) * 512])
                    nc.scalar.activation(outsb[:, j * 512:(j + 1) * 512], pso[:], AF.Copy)
                nc.sync.dma_start(out=out_d[:, t0:t0 + T], in_=outsb[:])

            # software pipeline: emit block k+1's front-end before block k's
            # scan loop so the in-order DVE queue never stalls at boundaries
            fe_cur = frontend(0)
            for blk in range(NBLK):
                fe_next = frontend(blk + 1) if blk + 1 < NBLK else None
                scan_block(fe_cur)
                fe_cur = fe_next
    nc.compile()
    return nc


def _get_nc():
    global _CACHED_NC
    if _CACHED_NC is None:
        _CACHED_NC = _build_nc()
    return _CACHED_NC


def _core_inputs(x_seq, p, half):
    """Per-core input dict. x_seq: (L, d_model) f32 (already flipped for bwd).
    p: dict of this direction's raw params. half: which d_inner half this
    core owns (own channels are always tile 0 / the 'own' slots)."""
    Win, convw, convb = p["Win"], p["convw"], p["convb"]
    Wx, Wdt, bdt, Alog, Dsk, Wout = p["Wx"], p["Wdt"], p["bdt"], p["Alog"], p["D"], p["Wout"]
    ln_g, ln_b = p["ln_g"], p["ln_b"]

    own = slice(half * EH, (half + 1) * EH)
    other = slice((1 - half) * EH, (2 - half) * EH)
    e_order = [own, other]

    Wg = Win * ln_g[None, :]                 # fold ln gain
    bvec = Win @ ln_b                        # fold ln bias
    Wx_in = Wg[0:D_INNER]
    bx_in = bvec[0:D_INNER]

    # conv folded weights: for tile slot s (0=own), tap k: diag(convw[:,k]) @ Win_xin
    wconvT = np.zeros((128, 2 * D_CONV * 128), np.float32)
    convb2 = np.zeros((128, 2), np.float32)
    for s, sl in enumerate(e_order):
        for k in range(D_CONV):
            Wk = convw[sl, k:k + 1] * Wx_in[sl, :]        # (128,128) = diag(w_k) @ W
            wconvT[:, (s * D_CONV + k) * 128:(s * D_CONV + k + 1) * 128] = Wk.T
        convb2[:, s] = convb[sl] + convw[sl].sum(1) * bx_in[sl]

    wzT = Wg[D_INNER + half * EH: D_INNER + (half + 1) * EH, :].T
    sbz = bvec[D_INNER + half * EH: D_INNER + (half + 1) * EH][:, None]

    wxT = np.concatenate([Wx[:, sl].T for sl in e_order], axis=1)   # (128, 80)
    wdtT = Wdt[own].T                                               # (8, 128)
    A = -np.exp(Alog[own])                                          # (128, 16)
    woutT = Wout[:, own].T                                          # (128, 128)

    return {
        "x": np.ascontiguousarray(x_seq, np.float32),
        "wconvT": wconvT.astype(np.float16),
        "wzT": np.ascontiguousarray(wzT, np.float16),
        "sbz": np.ascontiguousarray(sbz, np.float32),
        "convb": convb2,
        "wxT": np.ascontiguousarray(wxT, np.float16),
        "wdtT": np.ascontiguousarray(wdtT, np.float16),
        "bdt": np.ascontiguousarray(bdt[own][:, None], np.float32),
        "A": np.ascontiguousarray(A, np.float32),
        "Dskip": np.ascontiguousarray(Dsk[own][:, None], np.float32),
        "woutT": np.ascontiguousarray(woutT, np.float16),
        "ident": np.eye(128, dtype=np.float16),
    }


def kernel(**inputs):
    inputs = {k: np.asarray(v) for k, v in inputs.items()}
    x = inputs["x"].astype(np.float32)                       # (2,128,32,16,16)
    x_cl = x.reshape(B_SZ, D_MODEL, L)                       # (B, C, L)
    x_seq = x_cl.transpose(0, 2, 1)                          # (B, L, C)

    params = {}
    for s in ("f", "b"):
        params[s] = {
            "Win": inputs[f"Win_{s}"], "convw": inputs[f"convw_{s}"],
            "convb": inputs[f"convb_{s}"], "Wx": inputs[f"Wx_{s}"],
            "Wdt": inputs[f"Wdt_{s}"], "bdt": inputs[f"bdt_{s}"],
            "Alog": inputs[f"Alog_{s}"], "D": inputs[f"D_{s}"],
            "Wout": inputs[f"Wout_{s}"], "ln_g": inputs["ln_g"],
            "ln_b": inputs["ln_b"],
        }

    in_maps = []
    meta = []
    for b in range(B_SZ):
        for s in ("f", "b"):
            xs = x_seq[b] if s == "f" else x_seq[b, ::-1]
            for half in (0, 1):
                in_maps.append(_core_inputs(xs, params[s], half))
                meta.append((b, s))

    nc = _get_nc()
    res = run_bass_kernel_spmd(nc, in_maps, list(range(8)))

    acc = np.zeros((B_SZ, D_MODEL, L), np.float32)
    for i, (b, s) in enumerate(meta):
        o = res.results[i]["out"]                            # (d_model, L)
        if s == "b":
            o = o[:, ::-1]
        acc[b] += o
    out = x_cl + acc
    return out.reshape(x.shape).astype(np.float32)


# revision 19
# speedup vs baseline: 1.4143x; 1.1106x over previous
"""Bidirectional Mamba layer on 8 Trainium2 NeuronCores.

Sharding: core = (batch b in {0,1}) x (direction in {fwd,bwd}) x
(d_inner half in {0,1}).  Each core runs the full front-end (LN,
in_proj, conv, x_proj, dt) and the selective scan + output projection
for its 128 d_inner channels.  The host flips the sequence for the
backward direction, slices weights per core, and sums the 4 partial
(d_model, L) outputs per batch plus the residual.

One SPMD Bass graph serves all 8 cores; all per-core variation lives in
the input data (weight slices / flipped x).
"""

import math
import numpy as np

import concourse.bass as bass
import concourse.bacc as bacc
import concourse.mybir as mybir
from concourse import tile
from concourse.bass_utils import run_bass_kernel_spmd

# Problem shape (hardcoded per contract)
B_SZ = 2
D_MODEL = 128
D_STATE = 16
D_CONV = 4
EXPAND = 2
D_INNER = EXPAND * D_MODEL          # 256
DT_RANK = math.ceil(D_MODEL / 16)   # 8
LN_EPS = 1e-5
SPATIAL = (32, 16, 16)
L = 32 * 16 * 16                    # 8192
EH = 128                            # d_inner half per core
T = 2048                            # time block
NBLK = L // T
NT = T // 512                       # 512-tiles per block

f32 = mybir.dt.float32
f16 = mybir.dt.float16
A_OP = mybir.AluOpType
AF = mybir.ActivationFunctionType

_CACHED_NC = None


def _build_nc():
    nc = bacc.Bacc("TRN2", target_bir_lowering=False, debug=False, num_devices=8)

    # ---- DRAM parameters (per-core data) ----
    x_d = nc.declare_dram_parameter("x", [L, D_MODEL], f32, isOutput=False)
    wconv_d = nc.declare_dram_parameter("wconvT", [128, 2 * D_CONV * 128], f16, isOutput=False)
    wz_d = nc.declare_dram_parameter("wzT", [128, 128], f16, isOutput=False)
    sbz_d = nc.declare_dram_parameter("sbz", [128, 1], f32, isOutput=False)
    convb_d = nc.declare_dram_parameter("convb", [128, 2], f32, isOutput=False)
    wx_d = nc.declare_dram_parameter("wxT", [128, 80], f16, isOutput=False)
    wdt_d = nc.declare_dram_parameter("wdtT", [DT_RANK, 128], f16, isOutput=False)
    bdt_d = nc.declare_dram_parameter("bdt", [128, 1], f32, isOutput=False)
    a_d = nc.declare_dram_parameter("A", [128, D_STATE], f32, isOutput=False)
    dsk_d = nc.declare_dram_parameter("Dskip", [128, 1], f32, isOutput=False)
    wout_d = nc.declare_dram_parameter("woutT", [128, 128], f16, isOutput=False)
    ident_d = nc.declare_dram_parameter("ident", [128, 128], f16, isOutput=False)
    out_d = nc.declare_dram_parameter("out", [D_MODEL, L], f32, isOutput=True)

    with tile.TileContext(nc) as tc:
        with (
            tc.tile_pool(name="const", bufs=1) as cpool,
            tc.tile_pool(name="ln", bufs=3) as lnpool,
            tc.tile_pool(name="fe", bufs=2) as fepool,
            tc.tile_pool(name="scan", bufs=2) as spool,
            tc.tile_pool(name="bcast", bufs=4) as bpool,
            tc.tile_pool(name="scr1", bufs=1) as scrpool,
            tc.tile_pool(name="mm", bufs=3, space="PSUM") as mmpool,
            tc.tile_pool(name="psx", bufs=1, space="PSUM") as psxpool,
            tc.tile_pool(name="ypsum", bufs=1, space="PSUM") as ypool,
            tc.tile_pool(name="dram", bufs=2, space="DRAM") as dpool,
        ):
            # ---- constants ----
            wconv = cpool.tile([128, 2 * D_CONV * 128], f16)
            wz = cpool.tile([128, 128], f16)
            sbz = cpool.tile([128, 1], f32)
            convb = cpool.tile([128, 2], f32)
            wx = cpool.tile([128, 80], f16)
            wdt = cpool.tile([DT_RANK, 128], f16)
            bdt = cpool.tile([128, 1], f32)
            a_t = cpool.tile([128, D_STATE], f32)
            dsk = cpool.tile([128, 1], f32)
            wout = cpool.tile([128, 128], f16)
            ident = cpool.tile([128, 128], f16)
            carry = cpool.tile([128, D_STATE], f32)
            for sb_t, dr in ((wconv, wconv_d), (wz, wz_d), (sbz, sbz_d),
                             (convb, convb_d), (wx, wx_d), (wdt, wdt_d),
                             (bdt, bdt_d), (a_t, a_d), (dsk, dsk_d),
                             (wout, wout_d), (ident, ident_d)):
                nc.sync.dma_start(out=sb_t[:], in_=dr[:])
            nc.vector.memset(carry[:], 0.0)

            prev_xn_box = [None]

            def frontend(blk):
                prev_xn = prev_xn_box[0]
                t0 = blk * T
                # ---------- LN + transpose into xn (c-part, 3+T) ----------
                xn = fepool.tile([128, 3 + T], f16, tag="xn")
                if prev_xn is None:
                    nc.vector.memset(xn[:, 0:3], 0.0)
                else:
                    nc.vector.tensor_copy(xn[:, 0:3], prev_xn[:, T:T + 3])
                # LN: one block DMA, per-tile stats (Identity/Square live in
                # every act table), then ONE Ln + ONE Exp per block for
                # rsqrt = exp(-0.5*ln(v+eps)) — keeps everything on the
                # exp+ln table (same one softplus and the scan's exps use).
                xts = fepool.tile([128, 16, 128], f32, tag="xts")
                nc.sync.dma_start(
                    out=xts[:],
                    in_=x_d[t0:t0 + T, :].rearrange("(i p) c -> p i c", p=128))
                # stats via DVE reduces (no per-tile ScalarE round-trips):
                # m = mean(x), v = mean(x^2) - m^2, r = exp(-0.5*ln(v+eps))
                xsq = scrpool.tile([128, 16, 128], f32, tag="xsq")
                nc.scalar.activation(xsq[:], xts[:], AF.Square)
                s1r = lnpool.tile([128, 16], f32, tag="s1r")
                nc.vector.tensor_reduce(s1r[:], xts[:], mybir.AxisListType.X, A_OP.add)
                s2r = lnpool.tile([128, 16], f32, tag="s2r")
                nc.vector.tensor_reduce(s2r[:], xsq[:], mybir.AxisListType.X, A_OP.add)
                negm16 = lnpool.tile([128, 16], f32, tag="negm16")
                nc.vector.tensor_scalar(negm16[:], s1r[:], -1.0 / 128, None, A_OP.mult)
                m2 = lnpool.tile([128, 16], f32, tag="m2")
                nc.scalar.activation(m2[:], s1r[:], AF.Square, scale=1.0 / 128)
                v16 = lnpool.tile([128, 16], f32, tag="v16")
                nc.vector.tensor_scalar(v16[:], s2r[:], 1.0 / 128, LN_EPS, A_OP.mult, A_OP.add)
                nc.vector.tensor_tensor(v16[:], v16[:], m2[:], A_OP.subtract)
                lnv16 = lnpool.tile([128, 16], f32, tag="lnv16")
                nc.scalar.activation(lnv16[:], v16[:], AF.Ln)
                r16 = lnpool.tile([128, 16], f32, tag="r16")
                nc.scalar.activation(r16[:], lnv16[:], AF.Exp, scale=-0.5)
                for j in range(NT):
                    psx = psxpool.tile([128, 512], f16, tag="psx")
                    for q in range(4):
                        i = j * 4 + q
                        xnorm = lnpool.tile([128, 128], f16, tag="xnorm")
                        nc.vector.tensor_scalar(xnorm[:], xts[:, i, :], negm16[:, i:i + 1],
                                                r16[:, i:i + 1], A_OP.add, A_OP.mult)
                        nc.tensor.transpose(psx[:, q * 128:(q + 1) * 128], xnorm[:], ident[:])
                    nc.scalar.activation(xn[:, 3 + j * 512: 3 + (j + 1) * 512], psx[:], AF.Copy)

                # ---------- in_proj(z) + conv(in_proj(x)) + x_proj + dt ----------
                zs = fepool.tile([128, T], f16, tag="zs")
                xc0 = fepool.tile([128, T], f16, tag="xc0")   # own half
                xc1 = fepool.tile([128, T], f16, tag="xc1")
                dt_t = fepool.tile([128, T], f16, tag="dt")
                dtr = fepool.tile([DT_RANK, T], f16, tag="dtr")
                bc16 = fepool.tile([32, T], f16, tag="bc16")
                for j in range(NT):
                    w0 = 3 + j * 512
                    # z half
                    psz = mmpool.tile([128, 512], f32, tag="mm")
                    nc.tensor.matmul(psz[:], wz[:], xn[:, w0:w0 + 512])
                    nc.scalar.activation(zs[:, j * 512:(j + 1) * 512], psz[:], AF.Silu, bias=sbz[:])
                    # conv via 4 shifted matmuls per e-tile
                    for et, xc in ((0, xc0), (1, xc1)):
                        psc = mmpool.tile([128, 512], f32, tag="mm")
                        for k in range(D_CONV):
                            nc.tensor.matmul(
                                psc[:],
                                wconv[:, (et * D_CONV + k) * 128:(et * D_CONV + k + 1) * 128],
                                xn[:, w0 - 3 + k: w0 - 3 + k + 512],
                                start=(k == 0), stop=(k == D_CONV - 1))
                        nc.scalar.activation(xc[:, j * 512:(j + 1) * 512], psc[:],
                                             AF.Silu, bias=convb[:, et:et + 1])
                    # x_proj (contract both e-tiles; dt-rows and B/C-rows as
                    # separate matmuls so PSUM reads start at partition 0)
                    psdt_in = mmpool.tile([DT_RANK, 512], f32, tag="mm")
                    nc.tensor.matmul(psdt_in[:], wx[:, 0:8], xc0[:, j * 512:(j + 1) * 512],
                                     start=True, stop=False)
                    nc.tensor.matmul(psdt_in[:], wx[:, 40:48], xc1[:, j * 512:(j + 1) * 512],
                                     start=False, stop=True)
                    psbc = mmpool.tile([32, 512], f32, tag="mm")
                    nc.tensor.matmul(psbc[:], wx[:, 8:40], xc0[:, j * 512:(j + 1) * 512],
                                     start=True, stop=False)
                    nc.tensor.matmul(psbc[:], wx[:, 48:80], xc1[:, j * 512:(j + 1) * 512],
                                     start=False, stop=True)
                    nc.scalar.activation(dtr[:, j * 512:(j + 1) * 512],
                                         psdt_in[:], AF.Copy)
                    nc.scalar.activation(bc16[:, j * 512:(j + 1) * 512],
                                         psbc[:], AF.Copy)
                # dt = softplus(Wdt @ dtr + bdt) = ln(1 + exp(.)); exp pass
                # grouped, then one block-wide ln (keeps table loads rare)
                e1 = fepool.tile([128, T], f16, tag="e1")
                for j in range(NT):
                    psdt = mmpool.tile([128, 512], f32, tag="mm")
                    nc.tensor.matmul(psdt[:], wdt[:], dtr[:, j * 512:(j + 1) * 512])
                    nc.scalar.activation(e1[:, j * 512:(j + 1) * 512], psdt[:], AF.Exp, bias=bdt[:])
                nc.scalar.activation(dt_t[:], e1[:], AF.Ln, bias=1.0)

                dtx = fepool.tile([128, T], f16, tag="dtx")
                nc.vector.tensor_tensor(dtx[:], dt_t[:], xc0[:], A_OP.mult)

                # bounce B/C rows through DRAM for partition-broadcast reads
                bcd = dpool.tile([32, T], f16, tag="bcd")
                nc.sync.dma_start(out=bcd[:], in_=bc16[:])
                prev_xn_box[0] = xn
                return dict(t0=t0, zs=zs, xc0=xc0, dt_t=dt_t, dtx=dtx, bcd=bcd)

            def scan_block(fe):
                t0, zs, xc0, dt_t, dtx, bcd = (fe["t0"], fe["zs"], fe["xc0"],
                                               fe["dt_t"], fe["dtx"], fe["bcd"])
                # ---------- selective scan over 16 states ----------
                # y = sum_n ch_n accumulates in PSUM via identity matmuls
                # (TensorE), keeping the adds off the DVE.
                ypsum = ypool.tile([128, T], f32, tag="ypsum")
                for n in range(D_STATE):
                    da = spool.tile([128, T], f32, tag="da")
                    nc.scalar.activation(da[:], dt_t[:], AF.Exp, scale=a_t[:, n:n + 1])
                    bb = bpool.tile([128, T], f16, tag="bb")
                    nc.sync.dma_start(out=bb[:], in_=bcd[n:n + 1, :].partition_broadcast(128))
                    u = spool.tile([128, T], f16, tag="u")
                    nc.vector.tensor_tensor(u[:], dtx[:], bb[:], A_OP.mult)
                    h = spool.tile([128, T], f16, tag="h")
                    nc.vector.tensor_tensor_scan(h[:], da[:], u[:], carry[:, n:n + 1],
                                                 A_OP.mult, A_OP.add)
                    nc.scalar.activation(carry[:, n:n + 1], h[:, T - 1:T], AF.Copy)
                    cb = bpool.tile([128, T], f16, tag="cb")
                    nc.sync.dma_start(out=cb[:], in_=bcd[16 + n:17 + n, :].partition_broadcast(128))
                    ch = spool.tile([128, T], f16, tag="ch")
                    nc.vector.tensor_tensor(ch[:], h[:], cb[:], A_OP.mult)
                    for j in range(NT):
                        nc.tensor.matmul(ypsum[:, j * 512:(j + 1) * 512], ident[:],
                                         ch[:, j * 512:(j + 1) * 512],
                                         start=(n == 0), stop=(n == D_STATE - 1),
                                         skip_group_check=True)

                # ---------- D-skip + gate + out_proj ----------
                y2 = fepool.tile([128, T], f16, tag="y2")
                nc.vector.scalar_tensor_tensor(y2[:], xc0[:], dsk[:], ypsum[:],
                                               A_OP.mult, A_OP.add)
                nc.vector.tensor_tensor(y2[:], y2[:], zs[:], A_OP.mult)
                outsb = fepool.tile([128, T], f32, tag="outsb")
                for j in range(NT):
                    pso = mmpool.tile([128, 512], f32, tag="mm")
                    nc.tensor.matmul(pso[:], wout[:], y2[:, j * 512:(j + 1) * 512])
                    nc.scalar.activation(outsb[:, j * 512:(j + 1) * 512], pso[:], AF.Copy)
                nc.sync.dma_start(out=out_d[:, t0:t0 + T], in_=outsb[:])

            # software pipeline: emit block k+1's front-end before block k's
            # scan loop so the in-order DVE queue never stalls at boundaries
            fe_cur = frontend(0)
            for blk in range(NBLK):
                fe_next = frontend(blk + 1) if blk + 1 < NBLK else None
                scan_block(fe_cur)
                fe_cur = fe_next
    nc.compile()
    return nc


def _get_nc():
    global _CACHED_NC
    if _CACHED_NC is None:
        _CACHED_NC = _build_nc()
    return _CACHED_NC


def _core_inputs(x_seq, p, half):
    """Per-core input dict. x_seq: (L, d_model) f32 (already flipped for bwd).
    p: dict of this direction's raw params. half: which d_inner half this
    core owns (own channels are always tile 0 / the 'own' slots)."""
    Win, convw, convb = p["Win"], p["convw"], p["convb"]
    Wx, Wdt, bdt, Alog, Dsk, Wout = p["Wx"], p["Wdt"], p["bdt"], p["Alog"], p["D"], p["Wout"]
    ln_g, ln_b = p["ln_g"], p["ln_b"]

    own = slice(half * EH, (half + 1) * EH)
    other = slice((1 - half) * EH, (2 - half) * EH)
    e_order = [own, other]

    Wg = Win * ln_g[None, :]                 # fold ln gain
    bvec = Win @ ln_b                        # fold ln bias
    Wx_in = Wg[0:D_INNER]
    bx_in = bvec[0:D_INNER]

    # conv folded weights: for tile slot s (0=own), tap k: diag(convw[:,k]) @ Win_xin
    wconvT = np.zeros((128, 2 * D_CONV * 128), np.float32)
    convb2 = np.zeros((128, 2), np.float32)
    for s, sl in enumerate(e_order):
        for k in range(D_CONV):
            Wk = convw[sl, k:k + 1] * Wx_in[sl, :]        # (128,128) = diag(w_k) @ W
            wconvT[:, (s * D_CONV + k) * 128:(s * D_CONV + k + 1) * 128] = Wk.T
        convb2[:, s] = convb[sl] + convw[sl].sum(1) * bx_in[sl]

    wzT = Wg[D_INNER + half * EH: D_INNER + (half + 1) * EH, :].T
    sbz = bvec[D_INNER + half * EH: D_INNER + (half + 1) * EH][:, None]

    wxT = np.concatenate([Wx[:, sl].T for sl in e_order], axis=1)   # (128, 80)
    wdtT = Wdt[own].T                                               # (8, 128)
    A = -np.exp(Alog[own])                                          # (128, 16)
    woutT = Wout[:, own].T                                          # (128, 128)

    return {
        "x": np.ascontiguousarray(x_seq, np.float32),
        "wconvT": wconvT.astype(np.float16),
        "wzT": np.ascontiguousarray(wzT, np.float16),
        "sbz": np.ascontiguousarray(sbz, np.float32),
        "convb": convb2,
        "wxT": np.ascontiguousarray(wxT, np.float16),
        "wdtT": np.ascontiguousarray(wdtT, np.float16),
        "bdt": np.ascontiguousarray(bdt[own][:, None], np.float32),
        "A": np.ascontiguousarray(A, np.float32),
        "Dskip": np.ascontiguousarray(Dsk[own][:, None], np.float32),
        "woutT": np.ascontiguousarray(woutT, np.float16),
        "ident": np.eye(128, dtype=np.float16),
    }


def kernel(**inputs):
    inputs = {k: np.asarray(v) for k, v in inputs.items()}
    x = inputs["x"].astype(np.float32)                       # (2,128,32,16,16)
    x_cl = x.reshape(B_SZ, D_MODEL, L)                       # (B, C, L)
    x_seq = x_cl.transpose(0, 2, 1)                          # (B, L, C)

    params = {}
    for s in ("f", "b"):
        params[s] = {
            "Win": inputs[f"Win_{s}"], "convw": inputs[f"convw_{s}"],
            "convb": inputs[f"convb_{s}"], "Wx": inputs[f"Wx_{s}"],
            "Wdt": inputs[f"Wdt_{s}"], "bdt": inputs[f"bdt_{s}"],
            "Alog": inputs[f"Alog_{s}"], "D": inputs[f"D_{s}"],
            "Wout": inputs[f"Wout_{s}"], "ln_g": inputs["ln_g"],
            "ln_b": inputs["ln_b"],
        }

    in_maps = []
    meta = []
    for b in range(B_SZ):
        for s in ("f", "b"):
            xs = x_seq[b] if s == "f" else x_seq[b, ::-1]
            for half in (0, 1):
                in_maps.append(_core_inputs(xs, params[s], half))
                meta.append((b, s))

    nc = _get_nc()
    res = run_bass_kernel_spmd(nc, in_maps, list(range(8)))

    acc = np.zeros((B_SZ, D_MODEL, L), np.float32)
    for i, (b, s) in enumerate(meta):
        o = res.results[i]["out"]                            # (d_model, L)
        if s == "b":
            o = o[:, ::-1]
        acc[b] += o
    out = x_cl + acc
    return out.reshape(x.shape).astype(np.float32)
